# revision 20
# baseline (speedup 1.0000x reference)
"""Bipolar LIF neuron forward pass on 8 Trainium2 NeuronCores.

Reference semantics (all fp32, per element over [B, N, F], recurrence over T):
    V_t   = alpha * V'_{t-1} + x_t          (V'_{-1} = 0)
    pos_t = (V_t >= 1.0)                    -> out[..., :F]
    neg_t = (V_t <= -1.0)                   -> out[..., F:]
    V'_t  = V_t - (pos_t + neg_t)           (both spikes subtract exactly 1)

Sharding: data-parallel over B (8 batches -> 8 cores, no communication).
Per core the layout is [T, N, F] with N=1024 folded as 128 partitions, so
each timestep is a [128, 1024] fp32 SBUF tile.

Design notes (the kernel is DMA-bound; per core: 16.8 MB in, 4.2-8.4 MB out):
  * The recurrent state is the PRE-RESET potential V_t; the reset is applied
    at the top of the NEXT step's fused custom DVE op, so one 1x DVE op per
    step carries the whole recurrence (6 ALU stages, bit-exact):
        y_t = alpha * (y_{t-1} - 1{y>=1} - 1{y<=-1}) + x_t
  * Output encoding is a HYBRID chosen to balance DVE time against DMA time
    (both ~1.93 us/step):
      - elements [0, FC): one byte code = pos + 2*neg in {0,1,2} via a second
        4-stage custom DVE op (f32 out; a u8-out custom op crashes the
        device) + an Activation-engine f32->u8 convert copy;
      - elements [FC, W): two separate u8 planes, pos and neg, via two Pool
        tensor_scalar compares (Pool is otherwise idle; its ~1.45ns/elem is
        fine for this small slice).
    All stored values are exact {0,1,2} so the result is bit-exact.
  * Stores are batched 4 timesteps per DMA ([code|pos|neg] row = 1344 B/
    partition/step); loads are batched 2 timesteps per DMA with issuance
    alternating between the SP and ACT sequencers (gpsimd DMA would burn
    ~1us of Pool ENGINE time per descriptor set).
"""

import os
import sys

for _p in ("/opt/trn_rl_repo",):
    if _p not in sys.path and os.path.isdir(_p):
        sys.path.insert(0, _p)

from contextlib import ExitStack

import numpy as np

import concourse.bass as bass  # noqa: F401  (AP types come through tile/bacc)
import concourse.tile as tile
from concourse import bacc, mybir
from concourse.bass_utils import run_bass_kernel_spmd

B, T, N, F = 8, 32, 1024, 128
P = 128            # SBUF partitions
J = N // P         # n-rows folded into each partition's free dim
W = J * F          # free elems per step (1024)
FC = 672           # elems/partition encoded as 1-byte codes (DVE custom op)
FP = W - FC        # elems/partition stored as separate pos/neg u8 planes
RB = FC + 2 * FP   # output row bytes per step (code | pos | neg)
SB = 4             # timesteps per output store batch
ALPHA = float(np.float32(np.exp(np.float32(-1.0 / 20.0))))

_NC_CACHE = {}


def _register_ops():
    """Two custom DVE ops, uops_sha self-pinned (lower() is deterministic).

    LIF_PRERESET_ANT: previous step's reset + this step's integrate.
        s   = (Src0 >= 1) + (Src0 <= -1)    [reset of the PREVIOUS V]
        out = (Src0 - s) * C0 + Src1        [alpha * V' + x = this step's V]
    Bit-exact vs the reference: y - s is exact fp32 (1 is a multiple of
    ulp(y) for |y| < 2^24) and the mult/add round identically.

    LIF_CODE3_ANT: spike code from the pre-reset V (C0 binds -1.0):
        out = (Src0 >= 1) + 2 * (Src0 <= C0)   in {0.0, 1.0, 2.0}
    """
    import concourse.dve_ops as dve_ops
    from concourse.dve_ops import DveOp, DveOpSpec
    from concourse.dve_spec import Spec, lower, Src0, Src1, C0, Zero, One, Latch

    def _add(name, spec, rd1):
        for o in dve_ops.OPS:
            if o.name == name:
                return o
        sha = DveOpSpec(name=name, opcode=0, uops=lower(spec, ver="v3"),
                        rd1_en=rd1).sha("v3")
        o = DveOp(name, spec, subdim=False, uops_sha={"v3": sha, "v4": "?"})
        dve_ops.OPS.append(o)
        dve_ops.CUSTOM_DVE_SPECS[name] = o.spec
        dve_ops._SUB_OPCODE_FOR_NAME[name] = (
            dve_ops._CUSTOM_DVE_ROW_BASE + len(dve_ops.OPS) - 1
        )
        return o

    LnOne = Latch(Zero - One)
    s1 = (Src0 >= One) + (Src0 <= LnOne)
    chain_body = (Src0 - s1) * C0 + Src1

    def _chain_ref(in0, in1, s0, s1_, imm2):
        v = in0.astype(np.float32)
        s = ((v >= np.float32(1.0)).astype(np.float32)
             + (v <= np.float32(-1.0)).astype(np.float32))
        q = (v - s).astype(np.float32)
        return (q * np.float32(s0)).astype(np.float32) + in1.astype(np.float32)

    lif = _add("LIF_PRERESET_ANT", Spec(body=chain_body, reference=_chain_ref),
               rd1=True)

    neg = Src0 <= C0
    code_body = (Src0 >= One) + (neg + neg)

    def _code_ref(in0, in1, s0, s1_, imm2):
        v = in0.astype(np.float32)
        return ((v >= np.float32(1.0)).astype(np.float32)
                + np.float32(2.0) * (v <= np.float32(s0)).astype(np.float32)
                ).astype(np.float32)

    code3 = _add("LIF_CODE3_ANT", Spec(body=code_body, reference=_code_ref),
                 rd1=False)
    return lif, code3


def _build_program():
    op = mybir.AluOpType
    AF = mybir.ActivationFunctionType
    f32 = mybir.dt.float32
    u8 = mybir.dt.uint8
    lif, code3 = _register_ops()

    nc = bacc.Bacc(
        "TRN2",
        target_bir_lowering=False,
        debug=False,
        enable_asserts=False,
    )
    # Input laid out host-side as [T/2, P, 2, W] so a 2-timestep load is one
    # aligned [P, 2*W] DMA (8 KiB contiguous per partition).
    x_d = nc.dram_tensor("x", [T // 2, P, 2, W], f32, kind="ExternalInput").ap()
    # Output batched SB timesteps per DMA; each step's row is
    # [FC code bytes | FP pos bytes | FP neg bytes].
    y_d = nc.dram_tensor("y", [T // SB, P, SB, RB], u8,
                         kind="ExternalOutput").ap()

    with tile.TileContext(nc) as tc, ExitStack() as ctx:
        xpool = ctx.enter_context(tc.tile_pool(name="xin", bufs=6))
        ypool = ctx.enter_context(tc.tile_pool(name="vstate", bufs=4))
        fpool = ctx.enter_context(tc.tile_pool(name="codef", bufs=4))
        cpool = ctx.enter_context(tc.tile_pool(name="out", bufs=3))

        y_prev = None
        out = None
        xtiles = {}
        ystate = {}

        def issue_load(t):
            # gpsimd = SWDGE: descriptor generation costs ~1us of Pool engine
            # but never parks a sequencer waiting on the shared DMA engines
            # (HWDGE queues stall in-order behind that wait). Loads are
            # EMITTED ~PF steps ahead of the compute that shares the Pool
            # queue, so a compute op waiting at the queue head never delays
            # the load stream (in-order queues).
            xt2 = xpool.tile([P, 2, W], f32, name="xt2")
            xtiles[t] = xt2
            if t == 0:
                # Split the first load per-step so the chain ramps ASAP.
                nc.gpsimd.dma_start(out=xt2[:, 0], in_=x_d[0][:, 0])
                nc.gpsimd.dma_start(out=xt2[:, 1], in_=x_d[0][:, 1])
            else:
                nc.gpsimd.dma_start(out=xt2[:], in_=x_d[t // 2])

        PF = 8  # prefetch distance in timesteps (even)
        for tp in range(0, PF, 2):
            issue_load(tp)
        for t in range(T):
            if t % 2 == 0 and t + PF < T:
                issue_load(t + PF)
            xt = xtiles[t - t % 2][:, t % 2]

            if t == 0:
                # V_0 = alpha*0 + x_0 = x_0: the loaded tile IS the state.
                yf = xt
            else:
                # Chain halves for t<=2 (ramp: start on half a step's data);
                # once code3 ops interleave between chain links the serial
                # dependency is >=2 ops old, so full-width links don't stall.
                yt = ypool.tile([P, W], f32, name="yt")
                if t <= 2:
                    for h0, h1 in ((0, W // 2), (W // 2, W)):
                        nc.vector._custom_dve(
                            lif, out=yt[:, h0:h1], in0=y_prev[:, h0:h1],
                            in1=xt[:, h0:h1], s0=ALPHA, s1=0.0
                        )
                else:
                    nc.vector._custom_dve(
                        lif, out=yt[:], in0=y_prev, in1=xt, s0=ALPHA, s1=0.0
                    )
                yf = yt[:]
            y_prev = yf
            ystate[t] = yf

            # Output-side ops are emitted ONE STEP LATE (for step t-1 during
            # iteration t): their producer then finished >=3 engine-ops ago,
            # so the in-order DVE/Pool engines never eat a sem-propagation
            # bubble waiting on the immediately preceding op.
            emit_for = [t - 1] if t > 0 else []
            if t == T - 1:
                emit_for.append(t)
            for tau in emit_for:
                ytau = ystate[tau]
                if tau % SB == 0:
                    out = cpool.tile([P, SB, RB], u8, name="out")
                i = tau % SB
                # Code region on DVE (custom op, f32 {0,1,2}) + ACT u8 convert.
                cfl = fpool.tile([P, FC], f32, name="cfl")
                nc.vector._custom_dve(code3, out=cfl[:], in0=ytau[:, 0:FC],
                                      s0=-1.0, s1=0.0)
                nc.scalar.activation(out[:, i, 0:FC], cfl[:], AF.Copy)
                # Plane region on Pool: pos and neg u8 compares.
                nc.gpsimd.tensor_scalar(out[:, i, FC:FC + FP], ytau[:, FC:W],
                                        1.0, None, op.is_ge)
                nc.gpsimd.tensor_scalar(out[:, i, FC + FP:RB], ytau[:, FC:W],
                                        -1.0, None, op.is_le)
                # The last batch is stored in [2,1,1]-step pieces so the
                # final store's transfer (and its ~1.5us issue latency) isn't
                # a 4-step-sized lump exposed after the last compute.
                last_batch = tau // SB == T // SB - 1
                if not last_batch:
                    if i == SB - 1:
                        nc.sync.dma_start(out=y_d[tau // SB], in_=out[:])
                else:
                    # Tail: store each step separately, alternating the SP and
                    # ACT queues so the final stores issue in parallel instead
                    # of serializing ~1us apart on one queue.
                    qeng = nc.sync if i % 2 == 0 else nc.scalar
                    qeng.dma_start(out=y_d[tau // SB][:, i:i + 1],
                                   in_=out[:, i:i + 1])

    nc.compile()
    return nc


def get_program():
    if "nc" not in _NC_CACHE:
        _NC_CACHE["nc"] = _build_program()
    return _NC_CACHE["nc"]


def kernel(input_current: np.ndarray, _return_bench=False, **_bench_kwargs):
    assert input_current.shape == (B, T, N, F), input_current.shape
    xs = np.ascontiguousarray(input_current, dtype=np.float32).reshape(
        B, T // 2, 2, P, W).transpose(0, 1, 3, 2, 4)
    xs = np.ascontiguousarray(xs)
    in_maps = [{"x": xs[b]} for b in range(B)]
    nc = get_program()
    res = run_bass_kernel_spmd(nc, in_maps, core_ids=list(range(B)), **_bench_kwargs)
    # Decode: first FC bytes are {0,1,2} codes, then FP pos bytes, FP neg
    # bytes. Widen to the reference's [B, T, N, 2F] float32 layout.
    out = np.empty((B, T, N, 2 * F), dtype=np.float32)
    pos = np.empty((T, P, W), dtype=np.float32)
    neg = np.empty((T, P, W), dtype=np.float32)
    for b in range(B):
        yb = res.results[b]["y"]  # [T/SB, P, SB, RB] u8
        yb = yb.transpose(0, 2, 1, 3).reshape(T, P, RB)
        code = yb[:, :, 0:FC]
        pos[:, :, 0:FC] = code == 1
        neg[:, :, 0:FC] = code == 2
        pos[:, :, FC:W] = yb[:, :, FC:FC + FP]
        neg[:, :, FC:W] = yb[:, :, FC + FP:RB]
        out[b, :, :, :F] = pos.reshape(T, N, F)
        out[b, :, :, F:] = neg.reshape(T, N, F)
    if _return_bench:
        return out, res
    return out


if __name__ == "__main__":
    x = np.random.randn(B, T, N, F).astype(np.float32)
    y = kernel(x)
    print("kernel output:", y.shape, y.dtype, "mean", y.mean())


# revision 24
# speedup vs baseline: 1.0476x; 1.0476x over previous
"""Bipolar LIF neuron forward pass on 8 Trainium2 NeuronCores.

Reference semantics (all fp32, per element over [B, N, F], recurrence over T):
    V_t   = alpha * V'_{t-1} + x_t          (V'_{-1} = 0)
    pos_t = (V_t >= 1.0)                    -> out[..., :F]
    neg_t = (V_t <= -1.0)                   -> out[..., F:]
    V'_t  = V_t - (pos_t + neg_t)           (both spikes subtract exactly 1)

Sharding: data-parallel over B (8 batches -> 8 cores, no communication).
Per core the layout is [T, N, F] with N=1024 folded as 128 partitions, so
each timestep is a [128, 1024] fp32 SBUF tile.

Design notes (the kernel is DMA-bound; per core: 16.8 MB in, 4.2-8.4 MB out):
  * The recurrent state is the PRE-RESET potential V_t; the reset is applied
    at the top of the NEXT step's fused custom DVE op, so one 1x DVE op per
    step carries the whole recurrence (6 ALU stages, bit-exact):
        y_t = alpha * (y_{t-1} - 1{y>=1} - 1{y<=-1}) + x_t
  * Output encoding is a HYBRID chosen to balance DVE time against DMA time
    (both ~1.93 us/step):
      - elements [0, FC): one byte code = pos + 2*neg in {0,1,2} via a second
        4-stage custom DVE op (f32 out; a u8-out custom op crashes the
        device) + an Activation-engine f32->u8 convert copy;
      - elements [FC, W): two separate u8 planes, pos and neg, via two Pool
        tensor_scalar compares (Pool is otherwise idle; its ~1.45ns/elem is
        fine for this small slice).
    All stored values are exact {0,1,2} so the result is bit-exact.
  * Stores are batched 4 timesteps per DMA ([code|pos|neg] row = 1344 B/
    partition/step); loads are batched 2 timesteps per DMA with issuance
    alternating between the SP and ACT sequencers (gpsimd DMA would burn
    ~1us of Pool ENGINE time per descriptor set).
"""

import os
import sys

for _p in ("/opt/trn_rl_repo",):
    if _p not in sys.path and os.path.isdir(_p):
        sys.path.insert(0, _p)

from contextlib import ExitStack

import numpy as np

import concourse.bass as bass  # noqa: F401  (AP types come through tile/bacc)
import concourse.tile as tile
from concourse import bacc, mybir
from concourse.bass_utils import run_bass_kernel_spmd

B, T, N, F = 8, 32, 1024, 128
P = 128            # SBUF partitions
J = N // P         # n-rows folded into each partition's free dim
W = J * F          # free elems per step (1024)
FC = 576           # elems/partition encoded as 1-byte codes (DVE custom op)
FP = W - FC        # elems/partition stored as separate pos/neg u8 planes
RB = FC + 2 * FP   # output row bytes per step (code | pos | neg)
SB = 4             # timesteps per output store batch
ALPHA = float(np.float32(np.exp(np.float32(-1.0 / 20.0))))

_NC_CACHE = {}


def _register_ops():
    """Two custom DVE ops, uops_sha self-pinned (lower() is deterministic).

    LIF_PRERESET_ANT: previous step's reset + this step's integrate.
        s   = (Src0 >= 1) + (Src0 <= -1)    [reset of the PREVIOUS V]
        out = (Src0 - s) * C0 + Src1        [alpha * V' + x = this step's V]
    Bit-exact vs the reference: y - s is exact fp32 (1 is a multiple of
    ulp(y) for |y| < 2^24) and the mult/add round identically.

    LIF_CODE3_ANT: spike code from the pre-reset V (C0 binds -1.0):
        out = (Src0 >= 1) + 2 * (Src0 <= C0)   in {0.0, 1.0, 2.0}
    """
    import concourse.dve_ops as dve_ops
    from concourse.dve_ops import DveOp, DveOpSpec
    from concourse.dve_spec import Spec, lower, Src0, Src1, C0, Zero, One, Latch

    def _add(name, spec, rd1):
        for o in dve_ops.OPS:
            if o.name == name:
                return o
        sha = DveOpSpec(name=name, opcode=0, uops=lower(spec, ver="v3"),
                        rd1_en=rd1).sha("v3")
        o = DveOp(name, spec, subdim=False, uops_sha={"v3": sha, "v4": "?"})
        dve_ops.OPS.append(o)
        dve_ops.CUSTOM_DVE_SPECS[name] = o.spec
        dve_ops._SUB_OPCODE_FOR_NAME[name] = (
            dve_ops._CUSTOM_DVE_ROW_BASE + len(dve_ops.OPS) - 1
        )
        return o

    LnOne = Latch(Zero - One)
    s1 = (Src0 >= One) + (Src0 <= LnOne)
    chain_body = (Src0 - s1) * C0 + Src1

    def _chain_ref(in0, in1, s0, s1_, imm2):
        v = in0.astype(np.float32)
        s = ((v >= np.float32(1.0)).astype(np.float32)
             + (v <= np.float32(-1.0)).astype(np.float32))
        q = (v - s).astype(np.float32)
        return (q * np.float32(s0)).astype(np.float32) + in1.astype(np.float32)

    lif = _add("LIF_PRERESET_ANT", Spec(body=chain_body, reference=_chain_ref),
               rd1=True)

    neg = Src0 <= C0
    code_body = (Src0 >= One) + (neg + neg)

    def _code_ref(in0, in1, s0, s1_, imm2):
        v = in0.astype(np.float32)
        return ((v >= np.float32(1.0)).astype(np.float32)
                + np.float32(2.0) * (v <= np.float32(s0)).astype(np.float32)
                ).astype(np.float32)

    code3 = _add("LIF_CODE3_ANT", Spec(body=code_body, reference=_code_ref),
                 rd1=False)
    return lif, code3


def _build_program(fc=None, pf=8, halves_all=True, tail_mode="sp211", pe_mode=False):
    global FC, FP, RB
    if fc is not None:
        FC = fc
        FP = W - FC
        RB = FC + 2 * FP
    if pe_mode:
        if fc is None or fc < 512:
            FC = 512
        FP = W - FC          # PE-combined code region (1 byte/elem too)
        RB = W               # whole row is 1-byte codes
    op = mybir.AluOpType
    AF = mybir.ActivationFunctionType
    f32 = mybir.dt.float32
    bf16 = mybir.dt.bfloat16
    u8 = mybir.dt.uint8
    lif, code3 = _register_ops()

    nc = bacc.Bacc(
        "TRN2",
        target_bir_lowering=False,
        debug=False,
        enable_asserts=False,
    )
    # Input laid out host-side as [T/2, P, 2, W] so a 2-timestep load is one
    # aligned [P, 2*W] DMA (8 KiB contiguous per partition).
    x_d = nc.dram_tensor("x", [T // 2, P, 2, W], f32, kind="ExternalInput").ap()
    id_d = (nc.dram_tensor("ident", [P, P], bf16, kind="ExternalInput").ap()
            if pe_mode else None)
    # Output batched SB timesteps per DMA; each step's row is
    # [FC code bytes | FP pos bytes | FP neg bytes].
    y_d = nc.dram_tensor("y", [T // SB, P, SB, RB], u8,
                         kind="ExternalOutput").ap()

    with tile.TileContext(nc) as tc, ExitStack() as ctx:
        xpool = ctx.enter_context(tc.tile_pool(name="xin", bufs=6))
        ypool = ctx.enter_context(tc.tile_pool(name="vstate", bufs=4))
        fpool = ctx.enter_context(tc.tile_pool(name="codef", bufs=4))
        cpool = ctx.enter_context(tc.tile_pool(name="out", bufs=3))
        if pe_mode:
            bpool = ctx.enter_context(tc.tile_pool(name="bits", bufs=3))
            ipool = ctx.enter_context(tc.tile_pool(name="cst", bufs=1))
            pspool = ctx.enter_context(
                tc.tile_pool(name="ps", bufs=3, space="PSUM"))
            ident = ipool.tile([P, P], bf16)
            nc.sync.dma_start(out=ident[:], in_=id_d)

        y_prev = None
        out = None
        xtiles = {}
        ystate = {}

        def issue_load(t):
            # gpsimd = SWDGE: descriptor generation costs ~1us of Pool engine
            # but never parks a sequencer waiting on the shared DMA engines
            # (HWDGE queues stall in-order behind that wait). Loads are
            # EMITTED ~PF steps ahead of the compute that shares the Pool
            # queue, so a compute op waiting at the queue head never delays
            # the load stream (in-order queues).
            xt2 = xpool.tile([P, 2, W], f32, name="xt2")
            xtiles[t] = xt2
            if t == 0:
                # Split the first load per-step so the chain ramps ASAP.
                nc.gpsimd.dma_start(out=xt2[:, 0], in_=x_d[0][:, 0])
                nc.gpsimd.dma_start(out=xt2[:, 1], in_=x_d[0][:, 1])
            else:
                nc.gpsimd.dma_start(out=xt2[:], in_=x_d[t // 2])

        PF = pf  # prefetch distance in timesteps (even)
        for tp in range(0, PF, 2):
            issue_load(tp)
        for t in range(T):
            if t % 2 == 0 and t + PF < T:
                issue_load(t + PF)
            xt = xtiles[t - t % 2][:, t % 2]

            if t == 0:
                # V_0 = alpha*0 + x_0 = x_0: the loaded tile IS the state.
                yf = xt
            else:
                # Chain halves for t<=2 (ramp: start on half a step's data);
                # once code3 ops interleave between chain links the serial
                # dependency is >=2 ops old, so full-width links don't stall.
                yt = ypool.tile([P, W], f32, name="yt")
                if halves_all or t <= 2:
                    for h0, h1 in ((0, W // 2), (W // 2, W)):
                        nc.vector._custom_dve(
                            lif, out=yt[:, h0:h1], in0=y_prev[:, h0:h1],
                            in1=xt[:, h0:h1], s0=ALPHA, s1=0.0
                        )
                else:
                    nc.vector._custom_dve(
                        lif, out=yt[:], in0=y_prev, in1=xt, s0=ALPHA, s1=0.0
                    )
                yf = yt[:]
            y_prev = yf
            ystate[t] = yf

            # Output-side ops are emitted ONE STEP LATE (for step t-1 during
            # iteration t): their producer then finished >=3 engine-ops ago,
            # so the in-order DVE/Pool engines never eat a sem-propagation
            # bubble waiting on the immediately preceding op.
            emit_for = [t - 1] if t > 0 else []
            if t == T - 1:
                emit_for.append(t)
            for tau in emit_for:
                ytau = ystate[tau]
                if tau % SB == 0:
                    out = cpool.tile([P, SB, RB], u8, name="out")
                i = tau % SB
                # Code region on DVE (custom op, f32 {0,1,2}) + ACT u8 convert.
                cfl = fpool.tile([P, FC], f32, name="cfl")
                nc.vector._custom_dve(code3, out=cfl[:], in0=ytau[:, 0:FC],
                                      s0=-1.0, s1=0.0)
                nc.scalar.activation(out[:, i, 0:FC], cfl[:], AF.Copy)
                if pe_mode:
                    # Second code region via the PE: pos+2*neg accumulated in
                    # one PSUM bank with identity weights, ACT converts to u8.
                    pp = bpool.tile([P, FP], bf16, name="pp")
                    nc.gpsimd.tensor_scalar(pp[:], ytau[:, FC:W], 1.0, None,
                                            op.is_ge)
                    n2 = bpool.tile([P, FP], bf16, name="n2")
                    nc.gpsimd.tensor_scalar(n2[:], ytau[:, FC:W], -1.0, 2.0,
                                            op.is_le, op.mult)
                    ps = pspool.tile([P, FP], f32, name="ps")
                    nc.tensor.matmul(out=ps[:], lhsT=ident[:], rhs=pp[:],
                                     start=True, stop=False)
                    nc.tensor.matmul(out=ps[:], lhsT=ident[:], rhs=n2[:],
                                     start=False, stop=True)
                    nc.scalar.activation(out[:, i, FC:RB], ps[:], AF.Copy)
                else:
                    # Plane region on Pool: pos and neg u8 compares.
                    nc.gpsimd.tensor_scalar(out[:, i, FC:FC + FP],
                                            ytau[:, FC:W], 1.0, None, op.is_ge)
                    nc.gpsimd.tensor_scalar(out[:, i, FC + FP:RB],
                                            ytau[:, FC:W], -1.0, None, op.is_le)
                # The last batch is stored in [2,1,1]-step pieces so the
                # final store's transfer (and its ~1.5us issue latency) isn't
                # a 4-step-sized lump exposed after the last compute.
                last_batch = tau // SB == T // SB - 1
                if not last_batch:
                    if i == SB - 1:
                        nc.sync.dma_start(out=y_d[tau // SB], in_=out[:])
                else:
                    # Tail: store each step separately, alternating the SP and
                    # ACT queues so the final stores issue in parallel instead
                    # of serializing ~1us apart on one queue.
                    if tail_mode == "alt":
                        qeng = nc.sync if i % 2 == 0 else nc.scalar
                        qeng.dma_start(out=y_d[tau // SB][:, i:i + 1],
                                       in_=out[:, i:i + 1])
                    elif tail_mode == "sp211":
                        if i == 1:
                            nc.sync.dma_start(out=y_d[tau // SB][:, 0:2],
                                              in_=out[:, 0:2])
                        elif i >= 2:
                            nc.sync.dma_start(out=y_d[tau // SB][:, i:i + 1],
                                              in_=out[:, i:i + 1])
                    else:
                        if i == SB - 1:
                            nc.sync.dma_start(out=y_d[tau // SB], in_=out[:])

    nc.compile()
    return nc


def get_program():
    if "nc" not in _NC_CACHE:
        _NC_CACHE["nc"] = _build_program()
    return _NC_CACHE["nc"]


def kernel(input_current: np.ndarray, _return_bench=False, **_bench_kwargs):
    assert input_current.shape == (B, T, N, F), input_current.shape
    xs = np.ascontiguousarray(input_current, dtype=np.float32).reshape(
        B, T // 2, 2, P, W).transpose(0, 1, 3, 2, 4)
    xs = np.ascontiguousarray(xs)
    in_maps = [{"x": xs[b]} for b in range(B)]
    nc = get_program()
    res = run_bass_kernel_spmd(nc, in_maps, core_ids=list(range(B)), **_bench_kwargs)
    # Decode: first FC bytes are {0,1,2} codes, then FP pos bytes, FP neg
    # bytes. Widen to the reference's [B, T, N, 2F] float32 layout.
    out = np.empty((B, T, N, 2 * F), dtype=np.float32)
    pos = np.empty((T, P, W), dtype=np.float32)
    neg = np.empty((T, P, W), dtype=np.float32)
    for b in range(B):
        yb = res.results[b]["y"]  # [T/SB, P, SB, RB] u8
        yb = yb.transpose(0, 2, 1, 3).reshape(T, P, RB)
        code = yb[:, :, 0:FC]
        pos[:, :, 0:FC] = code == 1
        neg[:, :, 0:FC] = code == 2
        pos[:, :, FC:W] = yb[:, :, FC:FC + FP]
        neg[:, :, FC:W] = yb[:, :, FC + FP:RB]
        out[b, :, :, :F] = pos.reshape(T, N, F)
        out[b, :, :, F:] = neg.reshape(T, N, F)
    if _return_bench:
        return out, res
    return out


if __name__ == "__main__":
    x = np.random.randn(B, T, N, F).astype(np.float32)
    y = kernel(x)
    print("kernel output:", y.shape, y.dtype, "mean", y.mean())
